# revision 34
# baseline (speedup 1.0000x reference)
"""MoE gating kernel for Trainium2 (Bass/Tile), data-parallel over 8 NeuronCores.

Computes: logits = x @ W_g.T ; top-2 values; softmax over the 2 values.
  p1 = sigmoid(v1 - v2), p2 = sigmoid(v2 - v1)  (v1 >= v2 the top-2 logits)

Sharding: tokens split 8 ways (2048 tokens/core), W_g replicated.

Measured ~60.5-61us typical (median ~61us; occasional 65-67us runs from
free-running clock-gate phase). Previous version of this kernel: 73.2us.

Design (every perturbation of the ring layout below measured worse —
adding a 5th/6th HWDGE tile or moving the constants off the pool ring
starves the pool's early tiles, which are the binding arrivals):
  - Stream: the 16 SDMA engines process one 8KB fp32 row per ~306ns each;
    2048 rows = ~39.2us aggregate on any DGE path, engines round-robin
    active queues fairly — only ordering/start are controllable. Engine
    FIFOs execute in fixed order, so an op gated on a late arrival blocks
    everything behind it; HWDGE has 8 DMAHW completion lanes and a 9th
    in-flight DMA stalls the issuing sequencer.
  - The SWDGE pool ring carries 12 tiles (in-order ~2.5us/tile cadence
    once solo; first descriptor ~9us after the Q7 boot); HWDGE carries 4
    early tiles that bridge the boot gap. Process order is the expected
    arrival order [0,1,2,14,3,15,4..13] (host unpermutes). The last two
    pool tiles stream as four d-quarters each (one per transpose quad,
    separate buffers): the piece-by-piece arrivals drip PE work through
    the tail window so the clock gate can't re-throttle there, and the
    final arrival gates only a quarter's transposes + mains.
  - The 4 HWDGE fp32 tiles are cast to bf16 whole-tile (2 DVE / 2 ACT) at
    FIFO positions matching their arrival.
  - Compute is fully per-tile (no group barrier): 16 bf16 transposes vs
    identity (regular matmuls: HAM-visible), drains split DVE/ACT, then
    16 x-STATIONARY mains (xT slice stationary, wgT moving) landing
    token-major logits [128t, 64e] in PSUM; max8 reads PSUM directly.
  - Warm-up + keeper matmuls hold the PE clock gate through early
    arrival gaps; sigmoid ACT table preloaded early on a memset scratch;
    one batched sigmoid + one contiguous partition-major store dispatched
    from the scalar ring (no cross-engine hop after the sigmoid).
bf16 adds ~4e-3 relative error on the output probabilities (gate 2e-2).
"""

import sys

sys.path.insert(0, "/opt/trn_rl_repo")

from contextlib import ExitStack

import numpy as np
import ml_dtypes

import concourse.bass as bass
import concourse.bacc as bacc
import concourse.mybir as mybir
from concourse.tile import TileContext
from concourse.bass_utils import run_bass_kernel_spmd

TOKENS = 16384
DIM = 2048
E = 64  # num experts
NCORES = 8
TPC = TOKENS // NCORES  # tokens per core
P = 128
KT = DIM // P  # 16 contraction tiles
NB = TPC // P  # 16 token blocks (tiles) per core
NSP = 4  # the last pool tiles stream as NSP d-pieces (one per quad)
QD = DIM // NSP

F32 = mybir.dt.float32
BF16 = mybir.dt.bfloat16
N_WARM = 16

SYNC_TILES = (0, 14)  # HWDGE sync ring, ring order
ACT_TILES = (1, 15)  # HWDGE scalar ring, ring order
S_TILES = tuple(range(2, 14))  # SWDGE pool ring, ring order
SPLIT_TILES = frozenset({12, 13})  # pool tiles DMA'd as NSP d-pieces
CAST_DVE = frozenset({0, 14})  # whole-tile cast engine per HWDGE tile
# process order == expected arrival order; host unpermutes
PROC_ORDER = (0, 1, 2, 14, 3, 15, 4, 5, 6, 7, 8, 9, 10, 11, 12, 13)


def _emit(tc, ctx, x_ap, wgt_ap, idb_ap, out_ap):
    nc = tc.nc

    singles = ctx.enter_context(tc.tile_pool(name="singles", bufs=1))
    xtpool = ctx.enter_context(tc.tile_pool(name="xtpool", bufs=4))
    spool = ctx.enter_context(tc.tile_pool(name="spool", bufs=4))
    psum_t = ctx.enter_context(tc.tile_pool(name="psum_t", bufs=3, space="PSUM"))
    psum_l = ctx.enter_context(tc.tile_pool(name="psum_l", bufs=2, space="PSUM"))
    psum_f = ctx.enter_context(tc.tile_pool(name="psum_f", bufs=1, space="PSUM"))
    psum_w = ctx.enter_context(tc.tile_pool(name="psum_w", bufs=1, space="PSUM"))

    warm = singles.tile([P, P], BF16)
    warm_rhs = singles.tile([P, 4 * P], BF16)
    sig_scratch = singles.tile([1, 2], F32)
    nc.vector.memset(warm[:], 0.0)
    nc.vector.memset(warm_rhs[:], 0.0)
    nc.vector.memset(sig_scratch[:], 0.0)

    warm_flip = [False]

    def warm_mm():
        # alternate PSUM banks: back-to-back matmuls into ONE bank
        # serialize on the write-after-write; alternating sustains the
        # ~80% duty HAM needs to flip
        warm_flip[0] = not warm_flip[0]
        if warm_flip[0]:
            pw = psum_w.tile([P, 4 * P], F32, tag="warm_ps")
        else:
            pw = psum_f.tile([P, 4 * P], F32, tag="fin_ps")
        nc.tensor.matmul(pw[:], warm[:], warm_rhs[:])

    for _ in range(N_WARM):
        warm_mm()

    def keeper(n=1):
        for _ in range(n):
            warm_mm()

    xf32 = {}
    xb = {}
    for t in range(NB):
        if t in SPLIT_TILES:
            xb[t] = [
                singles.tile([P, QD], BF16, tag=f"xb{t}p{i}", name=f"xb{t}p{i}")
                for i in range(NSP)
            ]
        else:
            xb[t] = [singles.tile([P, DIM], BF16, tag=f"xb{t}", name=f"xb{t}")]
    for t in SYNC_TILES + ACT_TILES:
        xf32[t] = singles.tile([P, DIM], F32, tag=f"xf{t}", name=f"xf{t}")

    ident = singles.tile([P, P], BF16)
    wgT = singles.tile([P, KT, E], BF16)

    def xb_chunk(t, k):
        # the [128, 128] k-th d-chunk of tile t's bf16 buffer(s)
        if t in SPLIT_TILES:
            piece = xb[t][k // (KT // NSP)]
            kk = k % (KT // NSP)
            return piece[:, kk * P : (kk + 1) * P]
        return xb[t][0][:, k * P : (k + 1) * P]

    # pool ring: identity first (transposes need it), then x tiles in
    # process order with wgT slotted before the first mains need it
    nc.gpsimd.dma_start(out=ident[:], in_=idb_ap)
    nc.gpsimd.dma_start(out=wgT[:], in_=wgt_ap)
    for t in S_TILES:
        if t in SPLIT_TILES:
            for i in range(NSP):
                nc.gpsimd.dma_start(
                    out=xb[t][i][:],
                    in_=x_ap[t * P : (t + 1) * P, i * QD : (i + 1) * QD],
                )
        else:
            nc.gpsimd.dma_start(out=xb[t][0][:], in_=x_ap[t * P : (t + 1) * P, :])
    for t in SYNC_TILES:
        nc.sync.dma_start(out=xf32[t][:], in_=x_ap[t * P : (t + 1) * P, :])
    for t in ACT_TILES:
        nc.scalar.dma_start(out=xf32[t][:], in_=x_ap[t * P : (t + 1) * P, :])

    # per-process-position (v1-v2, v2-v1) accumulate here
    dd_all = singles.tile([P, NB, 2], F32)
    sig_preloaded = [False]

    def cast_tile(t):
        if t in CAST_DVE:
            nc.vector.tensor_copy(xb[t][0][:], xf32[t][:])
        else:
            nc.scalar.copy(xb[t][0][:], xf32[t][:])
        if not sig_preloaded[0]:
            sig_preloaded[0] = True
            nc.scalar.activation(
                sig_scratch[:], sig_scratch[:], mybir.ActivationFunctionType.Sigmoid
            )

    # HAM bridge: the warm-ups end ~10us but the first transposes start
    # ~14.3us (waiting x0's cast) — longer than HAM's 3.4us MID window.
    # These keepers use the identity (pool ring, arrives ~11.5us) as the
    # stationary, so they execute mid-gap and keep the PE-busy signal up.
    for _ in range(2):
        warm_flip[0] = not warm_flip[0]
        pw = (psum_w if warm_flip[0] else psum_f).tile(
            [P, 4 * P], F32, tag="warm_ps" if warm_flip[0] else "fin_ps"
        )
        nc.tensor.matmul(pw[:], ident[:], warm_rhs[:])

    for pos in range(NB):
        t = PROC_ORDER[pos]
        if t in xf32:
            cast_tile(t)
        if 2 <= pos < 8 and pos % 2 == 0:
            keeper(1)

        # 16 regular bf16 transposes vs identity -> xt_t [128d-slices, t]
        xt_t = xtpool.tile([P, KT * P], BF16)
        for q in range(KT // 4):
            pt = psum_t.tile([P, 4 * P], F32)
            for j in range(4):
                k = 4 * q + j
                nc.tensor.matmul(
                    pt[:, j * P : (j + 1) * P],
                    xb_chunk(t, k),
                    ident[:],
                )
            dst = xt_t[:, 4 * q * P : (4 * q + 4) * P]
            if q % 2 == 0:
                nc.vector.tensor_copy(dst, pt[:])
            else:
                nc.scalar.copy(dst, pt[:])

        # x-stationary mains: logits land token-major [128t, 64e] in PSUM
        fp = psum_l.tile([P, E], F32)
        for k in range(KT):
            nc.tensor.matmul(
                fp[:],
                xt_t[:, k * P : (k + 1) * P],
                wgT[:, k, :],
                start=(k == 0),
                stop=(k == KT - 1),
            )
        max8 = spool.tile([P, 8], F32)
        nc.vector.max(out=max8[:], in_=fp[:])
        nc.vector.tensor_sub(dd_all[:, pos, 0:1], max8[:, 0:1], max8[:, 1:2])
        nc.vector.tensor_sub(dd_all[:, pos, 1:2], max8[:, 1:2], max8[:, 0:1])

    # single sigmoid + one contiguous partition-major store
    ot = singles.tile([P, NB, 2], F32)
    nc.scalar.activation(ot[:], dd_all[:], mybir.ActivationFunctionType.Sigmoid)
    nc.scalar.dma_start(out=out_ap, in_=ot[:])


_NC_CACHE = {}


def _build():
    key = "nc"
    if key in _NC_CACHE:
        return _NC_CACHE[key]
    nc = bacc.Bacc(trn_type="TRN2")
    x = nc.dram_tensor("x", [TPC, DIM], F32, kind="ExternalInput")
    wgt = nc.dram_tensor("wgt", [P, KT * E], BF16, kind="ExternalInput")
    idb = nc.dram_tensor("idb", [P, P], BF16, kind="ExternalInput")
    out = nc.dram_tensor("out", [P, NB * 2], F32, kind="ExternalOutput")
    with TileContext(nc) as tc, ExitStack() as ctx:
        _emit(tc, ctx, x.ap(), wgt.ap(), idb.ap(), out.ap())
    if not nc.is_finalized():
        nc.finalize()
    _NC_CACHE[key] = nc
    return nc


def _run(x, W_g, trace=False):
    nc = _build()
    x = np.ascontiguousarray(np.asarray(x, dtype=np.float32))
    W_g = np.asarray(W_g, dtype=np.float32)
    # host-side weight layout prep: wgt[p, k*E + e] = W_g[e, k*128 + p]
    wgt = np.ascontiguousarray(
        W_g.reshape(E, KT, P).transpose(2, 1, 0).reshape(P, KT * E)
    ).astype(ml_dtypes.bfloat16)
    idb = np.eye(P, dtype=np.float32).astype(ml_dtypes.bfloat16)
    in_maps = [
        {
            "x": np.ascontiguousarray(x[c * TPC : (c + 1) * TPC]),
            "wgt": wgt,
            "idb": idb,
        }
        for c in range(NCORES)
    ]
    res = run_bass_kernel_spmd(nc, in_maps, core_ids=list(range(NCORES)), trace=trace)
    # device output is partition-major [128, 16, 2] in PROCESS order;
    # de-interleave + unpermute: out[PROC_ORDER[b]*128 + p] = res[p, b]
    inv = np.argsort(np.array(PROC_ORDER))
    outs = []
    for r in res.results:
        o = r["out"].reshape(P, NB, 2)[:, inv, :].transpose(1, 0, 2).reshape(TPC, 2)
        outs.append(o)
    out = np.ascontiguousarray(np.concatenate(outs, axis=0))
    return out, res


def kernel(x, W_g):
    out, _ = _run(x, W_g, trace=False)
    return out


def kernel_profiled(x, W_g, **_kw):
    out, res = _run(x, W_g, trace=True)
    return out, res


# revision 35
# speedup vs baseline: 1.1177x; 1.1177x over previous
"""MoE gating kernel for Trainium2 (Bass/Tile), data-parallel over 8 NeuronCores.

Computes: logits = x @ W_g.T ; top-2 values; softmax over the 2 values.
  p1 = sigmoid(v1 - v2), p2 = sigmoid(v2 - v1)  (v1 >= v2 the top-2 logits)

Sharding: tokens split 8 ways (2048 tokens/core), W_g replicated.

Measured ~60.5-61us typical (median ~61us; occasional 65-67us runs from
free-running clock-gate phase). Previous version of this kernel: 73.2us.

Design (every perturbation of the ring layout below measured worse —
adding a 5th/6th HWDGE tile or moving the constants off the pool ring
starves the pool's early tiles, which are the binding arrivals):
  - Stream: the 16 SDMA engines process one 8KB fp32 row per ~306ns each;
    2048 rows = ~39.2us aggregate on any DGE path, engines round-robin
    active queues fairly — only ordering/start are controllable. Engine
    FIFOs execute in fixed order, so an op gated on a late arrival blocks
    everything behind it; HWDGE has 8 DMAHW completion lanes and a 9th
    in-flight DMA stalls the issuing sequencer.
  - The SWDGE pool ring carries 12 tiles (in-order ~2.5us/tile cadence
    once solo; first descriptor ~9us after the Q7 boot); HWDGE carries 4
    early tiles that bridge the boot gap. Process order is the expected
    arrival order [0,1,2,14,3,15,4..13] (host unpermutes). The last two
    pool tiles stream as four d-quarters each (one per transpose quad,
    separate buffers): the piece-by-piece arrivals drip PE work through
    the tail window so the clock gate can't re-throttle there, and the
    final arrival gates only a quarter's transposes + mains.
  - The 4 HWDGE fp32 tiles are cast to bf16 whole-tile (2 DVE / 2 ACT) at
    FIFO positions matching their arrival.
  - Compute is fully per-tile (no group barrier): 16 bf16 transposes vs
    identity (regular matmuls: HAM-visible), drains split DVE/ACT, then
    16 x-STATIONARY mains (xT slice stationary, wgT moving) landing
    token-major logits [128t, 64e] in PSUM; max8 reads PSUM directly.
  - Warm-up + keeper matmuls hold the PE clock gate through early
    arrival gaps; sigmoid ACT table preloaded early on a memset scratch;
    one batched sigmoid + one contiguous partition-major store dispatched
    from the scalar ring (no cross-engine hop after the sigmoid).
bf16 adds ~4e-3 relative error on the output probabilities (gate 2e-2).
"""

import sys

sys.path.insert(0, "/opt/trn_rl_repo")

from contextlib import ExitStack

import numpy as np
import ml_dtypes

import concourse.bass as bass
import concourse.bacc as bacc
import concourse.mybir as mybir
from concourse.tile import TileContext
from concourse.bass_utils import run_bass_kernel_spmd

TOKENS = 16384
DIM = 2048
E = 64  # num experts
NCORES = 8
TPC = TOKENS // NCORES  # tokens per core
P = 128
KT = DIM // P  # 16 contraction tiles
NB = TPC // P  # 16 token blocks (tiles) per core
NSP = 4  # the last pool tiles stream as NSP d-pieces (one per quad)
QD = DIM // NSP

F32 = mybir.dt.float32
BF16 = mybir.dt.bfloat16
N_WARM = 16

SYNC_TILES = (0, 14)  # HWDGE sync ring, ring order
ACT_TILES = (1, 15)  # HWDGE scalar ring, ring order
S_TILES = tuple(range(2, 14))  # SWDGE pool ring, ring order
SPLIT_TILES = frozenset({12, 13})  # pool tiles DMA'd as NSP d-pieces
CAST_DVE = frozenset({0, 14})  # whole-tile cast engine per HWDGE tile
# process order == expected arrival order; host unpermutes
PROC_ORDER = (0, 1, 2, 14, 3, 15, 4, 5, 6, 7, 8, 9, 10, 11, 12, 13)


def _emit(tc, ctx, x_ap, wgt_ap, idb_ap, out_ap):
    nc = tc.nc

    singles = ctx.enter_context(tc.tile_pool(name="singles", bufs=1))
    xtpool = ctx.enter_context(tc.tile_pool(name="xtpool", bufs=4))
    spool = ctx.enter_context(tc.tile_pool(name="spool", bufs=4))
    psum_t = ctx.enter_context(tc.tile_pool(name="psum_t", bufs=3, space="PSUM"))
    psum_l = ctx.enter_context(tc.tile_pool(name="psum_l", bufs=2, space="PSUM"))
    psum_f = ctx.enter_context(tc.tile_pool(name="psum_f", bufs=1, space="PSUM"))
    psum_w = ctx.enter_context(tc.tile_pool(name="psum_w", bufs=1, space="PSUM"))

    warm = singles.tile([P, P], BF16)
    warm_rhs = singles.tile([P, 4 * P], BF16)
    sig_scratch = singles.tile([1, 2], F32)
    nc.vector.memset(warm[:], 0.0)
    nc.vector.memset(warm_rhs[:], 0.0)
    nc.vector.memset(sig_scratch[:], 0.0)

    warm_flip = [False]

    def warm_mm():
        # alternate PSUM banks: back-to-back matmuls into ONE bank
        # serialize on the write-after-write; alternating sustains the
        # ~80% duty HAM needs to flip
        warm_flip[0] = not warm_flip[0]
        if warm_flip[0]:
            pw = psum_w.tile([P, 4 * P], F32, tag="warm_ps")
        else:
            pw = psum_f.tile([P, 4 * P], F32, tag="fin_ps")
        nc.tensor.matmul(pw[:], warm[:], warm_rhs[:])

    for _ in range(N_WARM):
        warm_mm()

    def keeper(n=1):
        for _ in range(n):
            warm_mm()

    xf32 = {}
    xb = {}
    for t in range(NB):
        if t in SPLIT_TILES:
            xb[t] = [
                singles.tile([P, QD], BF16, tag=f"xb{t}p{i}", name=f"xb{t}p{i}")
                for i in range(NSP)
            ]
        else:
            xb[t] = [singles.tile([P, DIM], BF16, tag=f"xb{t}", name=f"xb{t}")]
    for t in SYNC_TILES + ACT_TILES:
        xf32[t] = singles.tile([P, DIM], F32, tag=f"xf{t}", name=f"xf{t}")

    ident = singles.tile([P, P], BF16)
    wgT = singles.tile([P, KT, E], BF16)

    def xb_chunk(t, k):
        # the [128, 128] k-th d-chunk of tile t's bf16 buffer(s)
        if t in SPLIT_TILES:
            piece = xb[t][k // (KT // NSP)]
            kk = k % (KT // NSP)
            return piece[:, kk * P : (kk + 1) * P]
        return xb[t][0][:, k * P : (k + 1) * P]

    # pool ring: identity first (transposes need it), then x tiles in
    # process order with wgT slotted before the first mains need it
    nc.gpsimd.dma_start(out=ident[:], in_=idb_ap)
    nc.gpsimd.dma_start(out=wgT[:], in_=wgt_ap)
    for t in S_TILES:
        if t in SPLIT_TILES:
            for i in range(NSP):
                nc.gpsimd.dma_start(
                    out=xb[t][i][:],
                    in_=x_ap[t * P : (t + 1) * P, i * QD : (i + 1) * QD],
                )
        else:
            nc.gpsimd.dma_start(out=xb[t][0][:], in_=x_ap[t * P : (t + 1) * P, :])
    for t in SYNC_TILES:
        nc.sync.dma_start(out=xf32[t][:], in_=x_ap[t * P : (t + 1) * P, :])
    for t in ACT_TILES:
        nc.scalar.dma_start(out=xf32[t][:], in_=x_ap[t * P : (t + 1) * P, :])

    # per-process-position (v1-v2, v2-v1) accumulate here
    dd_all = singles.tile([P, NB, 2], F32)
    sig_preloaded = [False]

    def cast_tile(t):
        if t in CAST_DVE:
            nc.vector.tensor_copy(xb[t][0][:], xf32[t][:])
        else:
            nc.scalar.copy(xb[t][0][:], xf32[t][:])
        if not sig_preloaded[0]:
            sig_preloaded[0] = True
            nc.scalar.activation(
                sig_scratch[:], sig_scratch[:], mybir.ActivationFunctionType.Sigmoid
            )

    # HAM bridge: the warm-ups end ~10us but the first transposes start
    # ~14.3us (waiting x0's cast) — longer than HAM's 3.4us MID window.
    # These keepers use the identity (pool ring, arrives ~11.5us) as the
    # stationary, so they execute mid-gap and keep the PE-busy signal up.
    for _ in range(2):
        warm_flip[0] = not warm_flip[0]
        pw = (psum_w if warm_flip[0] else psum_f).tile(
            [P, 4 * P], F32, tag="warm_ps" if warm_flip[0] else "fin_ps"
        )
        nc.tensor.matmul(pw[:], ident[:], warm_rhs[:])

    for pos in range(NB):
        t = PROC_ORDER[pos]
        if t in xf32:
            cast_tile(t)
        if 2 <= pos < 8 and pos % 2 == 0:
            keeper(1)

        # 16 regular bf16 transposes vs identity -> xt_t [128d-slices, t]
        xt_t = xtpool.tile([P, KT * P], BF16)
        for q in range(KT // 4):
            pt = psum_t.tile([P, 4 * P], F32)
            for j in range(4):
                k = 4 * q + j
                nc.tensor.matmul(
                    pt[:, j * P : (j + 1) * P],
                    xb_chunk(t, k),
                    ident[:],
                )
            dst = xt_t[:, 4 * q * P : (4 * q + 4) * P]
            if t in SPLIT_TILES:
                # tail tiles: halve the drain latency on the critical
                # chain by splitting each quad's drain across DVE+ACT
                nc.vector.tensor_copy(dst[:, 0 : 2 * P], pt[:, 0 : 2 * P])
                nc.scalar.copy(dst[:, 2 * P :], pt[:, 2 * P :])
            elif q % 2 == 0:
                nc.vector.tensor_copy(dst, pt[:])
            else:
                nc.scalar.copy(dst, pt[:])

        # x-stationary mains: logits land token-major [128t, 64e] in PSUM
        fp = psum_l.tile([P, E], F32)
        for k in range(KT):
            nc.tensor.matmul(
                fp[:],
                xt_t[:, k * P : (k + 1) * P],
                wgT[:, k, :],
                start=(k == 0),
                stop=(k == KT - 1),
            )
        max8 = spool.tile([P, 8], F32)
        nc.vector.max(out=max8[:], in_=fp[:])
        nc.vector.tensor_sub(dd_all[:, pos, 0:1], max8[:, 0:1], max8[:, 1:2])
        nc.vector.tensor_sub(dd_all[:, pos, 1:2], max8[:, 1:2], max8[:, 0:1])

    # single sigmoid + one contiguous partition-major store
    ot = singles.tile([P, NB, 2], F32)
    nc.scalar.activation(ot[:], dd_all[:], mybir.ActivationFunctionType.Sigmoid)
    nc.scalar.dma_start(out=out_ap, in_=ot[:])


_NC_CACHE = {}


def _build():
    key = "nc"
    if key in _NC_CACHE:
        return _NC_CACHE[key]
    nc = bacc.Bacc(trn_type="TRN2")
    x = nc.dram_tensor("x", [TPC, DIM], F32, kind="ExternalInput")
    wgt = nc.dram_tensor("wgt", [P, KT * E], BF16, kind="ExternalInput")
    idb = nc.dram_tensor("idb", [P, P], BF16, kind="ExternalInput")
    out = nc.dram_tensor("out", [P, NB * 2], F32, kind="ExternalOutput")
    with TileContext(nc) as tc, ExitStack() as ctx:
        _emit(tc, ctx, x.ap(), wgt.ap(), idb.ap(), out.ap())
    if not nc.is_finalized():
        nc.finalize()
    _NC_CACHE[key] = nc
    return nc


def _run(x, W_g, trace=False):
    nc = _build()
    x = np.ascontiguousarray(np.asarray(x, dtype=np.float32))
    W_g = np.asarray(W_g, dtype=np.float32)
    # host-side weight layout prep: wgt[p, k*E + e] = W_g[e, k*128 + p]
    wgt = np.ascontiguousarray(
        W_g.reshape(E, KT, P).transpose(2, 1, 0).reshape(P, KT * E)
    ).astype(ml_dtypes.bfloat16)
    idb = np.eye(P, dtype=np.float32).astype(ml_dtypes.bfloat16)
    in_maps = [
        {
            "x": np.ascontiguousarray(x[c * TPC : (c + 1) * TPC]),
            "wgt": wgt,
            "idb": idb,
        }
        for c in range(NCORES)
    ]
    res = run_bass_kernel_spmd(nc, in_maps, core_ids=list(range(NCORES)), trace=trace)
    # device output is partition-major [128, 16, 2] in PROCESS order;
    # de-interleave + unpermute: out[PROC_ORDER[b]*128 + p] = res[p, b]
    inv = np.argsort(np.array(PROC_ORDER))
    outs = []
    for r in res.results:
        o = r["out"].reshape(P, NB, 2)[:, inv, :].transpose(1, 0, 2).reshape(TPC, 2)
        outs.append(o)
    out = np.ascontiguousarray(np.concatenate(outs, axis=0))
    return out, res


def kernel(x, W_g):
    out, _ = _run(x, W_g, trace=False)
    return out


def kernel_profiled(x, W_g, **_kw):
    out, res = _run(x, W_g, trace=True)
    return out, res
